# revision 62
# baseline (speedup 1.0000x reference)
"""Trainium2 Bass kernel for nn_BlipAttention_75007308857568.

Single-head BLIP attention: B=32, N=1024, C=768, fp32.
  qkv = x @ qkv_w + qkv_b ; q,k,v split
  scores = q @ k.T / sqrt(C) ; attn = softmax(scores)
  out = attn @ v
  y = (out.swapaxes(1,2).reshape(B,N,C)) @ proj_w + proj_b

Sharding: data-parallel over batch B across 8 NeuronCores (4 batches/core).

Per-core dataflow (transposed domain keeps contraction dims on SBUF
partitions):
  XT  = x[b].T                       (PE transposes, f32r identity ->
                                      1.5 cyc/row instead of fp32's 2)
  QT/KT = (Wq|Wk).T @ XT             (PE fp32r; per-partition qkv bias added
                                      on the ACT engine, output written
                                      directly as fp8e4)
  V   = x[b] @ Wv + v_bias           (PE fp32r; bias add + fp8e4 convert on
                                      the Pool engine)
  scoresT[m,n] = KT.T@QT             (PE fp8 DoubleRow: 256-deep contraction
                                      per instr at 0.5 cyc/row)
  expT = exp(scoresT/sqrt(C) - 2)    (ACT, PSUM->SBUF fp8e4; the -2 shift
                                      keeps exp <= e^4.8 < 240 = trn2 e4m3
                                      max; softmax is shift-invariant)
  denom = ones.T @ expT              (PE fp8 DoubleRow)
  OT[c,n] = (V.T @ expT) * recip     (PE fp8 DoubleRow + DVE normalize)
  scratch flat = OT (c-major)        -> flat viewed as [N,C] IS the
                                       swapaxes+reshape permutation for free
  PT = transpose(P rows)             (PE, bf16 identity)
  y = P @ proj_w + proj_b            (PE fp32r; bias add on Pool engine)

Engine balance: PE is the bottleneck; ACT takes the qk bias adds and exp
(per-partition bias), DVE takes everything else PSUM-sourced (GPSIMD/Pool
cannot access PSUM). Engine queues are in-order, so next-batch QKV matmuls
are interleaved into the attention emission stream to cover the PE bubbles
that the scores->exp->denom/AV dependency chain would otherwise create.
"""

import math
import os

import numpy as np

import concourse.bacc as bacc
import concourse.bass as bass
import concourse.mybir as mybir
import concourse.tile as tile

from concourse.bass_utils import run_bass_kernel_spmd
from concourse.masks import make_identity

B, N, C = 32, 1024, 768
NCORES = 8
BPC = B // NCORES  # batches per core
CB = C // 128      # 6 channel blocks
NB = N // 128      # 8 sequence blocks
NH = 512           # n-half width (PSUM bank / fp32 moving-operand limit)
SCALE = 1.0 / math.sqrt(C)
EXPB = -2.0        # exp shift (max logit ~6.73 -> exp(4.73)=113 < 240)

_CACHE = {}


def _build(mm_r: bool, fp8: bool):
    dt = mybir.dt
    MM = dt.float32r if mm_r else dt.float32
    AT = dt.float8e4 if fp8 else dt.bfloat16  # attention-core operand dtype
    f32 = dt.float32
    DR = mybir.MatmulPerfMode.DoubleRow if fp8 else None

    nc = bacc.Bacc("TRN2", target_bir_lowering=False, debug=False)

    # xs/scr are declared in the matmul dtype (f32r == f32 bits) so the PE
    # transposes can pair with the bf16 identity (1 cyc/row, not fp32's 2)
    xs = nc.dram_tensor("xs", [BPC, N, C], MM, kind="ExternalInput")
    qkv_w = nc.dram_tensor("qkv_w", [C, 3 * C], MM, kind="ExternalInput")
    qkv_b = nc.dram_tensor("qkv_b", [3 * C], f32, kind="ExternalInput")
    proj_w = nc.dram_tensor("proj_w", [C, C], MM, kind="ExternalInput")
    proj_b = nc.dram_tensor("proj_b", [C], f32, kind="ExternalInput")
    y = nc.dram_tensor("y", [BPC, N, C], f32, kind="ExternalOutput")

    with tile.TileContext(nc) as tc:
        with (
            tc.tile_pool(name="consts", bufs=1) as consts,
            tc.tile_pool(name="xt", bufs=1) as pool_xt,
            tc.tile_pool(name="qt", bufs=2) as pool_qt,
            tc.tile_pool(name="kt", bufs=2) as pool_kt,
            tc.tile_pool(name="v", bufs=1) as pool_v,
            tc.tile_pool(name="expt", bufs=1) as pool_expt,
            tc.tile_pool(name="row", bufs=4) as pool_row,
            tc.tile_pool(name="pt", bufs=2) as pool_pt,
            tc.tile_pool(name="rb", bufs=2) as pool_rb,
            tc.tile_pool(name="scr", bufs=2, space="DRAM") as pool_scr,
            tc.tile_pool(name="psmm", bufs=6, space="PSUM") as psmm,
            tc.tile_pool(name="pst", bufs=2, space="PSUM") as pst,
        ):
            # ---- constants / weights (loaded once) ----
            # f32r identity: transpose cost keys on the moving operand (the
            # identity) — 1.5 cyc/row vs fp32's 2.0. bf16 would be 1.0 but
            # walrus forbids mixing 32-bit data with non-32-bit identity.
            # Built as f32 (memset/affine_select can't emit f32r), then
            # copied through DVE so the result is "rounded to f32r" as the
            # BIR verifier requires of f32r matmult inputs.
            ident_f = consts.tile([128, 128], f32, tag="ident_f")
            make_identity(nc, ident_f)
            ident = consts.tile([128, 128], MM, tag="ident")
            nc.vector.tensor_copy(ident, ident_f)

            W = consts.tile([128, CB, 3 * C], MM, tag="W")
            PW = consts.tile([128, CB, C], MM, tag="PW")

            # q/k bias as per-partition scalars: qkb[p, ob] = qkv_b[ob*128+p]
            qkb = consts.tile([128, 2 * CB], f32, tag="qkb")
            nc.sync.dma_start(
                qkb, qkv_b.ap()[0 : 2 * C].rearrange("(ob p) -> p ob", p=128)
            )
            # v bias / proj bias replicated across partitions
            vb = consts.tile([128, C], f32, tag="vb")
            nc.sync.dma_start(vb, qkv_b.ap()[None, 2 * C : 3 * C].to_broadcast([128, C]))
            pb = consts.tile([128, C], f32, tag="pb")
            nc.sync.dma_start(pb, proj_b.ap()[None, :].to_broadcast([128, C]))

            expb = consts.tile([128, 1], f32, tag="expb")
            nc.gpsimd.memset(expb, EXPB)

            # all-ones stationary operand for the softmax denominator matmul
            # (replicates the column sums of expT onto all 128 partitions)
            ones_f = consts.tile([128, 256 if fp8 else 128], f32, tag="ones_f")
            nc.gpsimd.memset(ones_f, 1.0)
            ones = consts.tile([128, 256 if fp8 else 128], AT, tag="ones")
            nc.gpsimd.tensor_copy(ones, ones_f)

            def transpose_block(src_row, dst_slices):
                """PE-transpose six 128x128 chunks of src_row, batched 4+2
                per PSUM bank, with one grouped DVE copy per bank."""
                psA = pst.tile([128, NH], MM, tag="tp4")
                for k in range(4):
                    nc.tensor.transpose(
                        psA[:, k * 128 : (k + 1) * 128],
                        src_row[:, k * 128 : (k + 1) * 128],
                        ident,
                    )
                nc.vector.tensor_copy(
                    dst_slices[0], psA.rearrange("p (c k) -> p c k", k=128)
                )
                psB = pst.tile([128, NH], MM, tag="tp4")
                for k in range(2):
                    nc.tensor.transpose(
                        psB[:, k * 128 : (k + 1) * 128],
                        src_row[:, (4 + k) * 128 : (5 + k) * 128],
                        ident,
                    )
                nc.vector.tensor_copy(
                    dst_slices[1],
                    psB[:, 0:256].rearrange("p (c k) -> p c k", k=128),
                )

            def emit_a_row_dma(b, nb):
                xrow = pool_row.tile([128, C], MM, tag="row")
                nc.sync.dma_start(xrow, xs.ap()[b, nb * 128 : (nb + 1) * 128, :])
                return xrow

            def emit_a_row_transpose(XT, xrow, nb):
                nsl = slice(nb * 128, (nb + 1) * 128)
                transpose_block(xrow, [XT[:, 0:4, nsl], XT[:, 4:6, nsl]])

            def stage_a(b):
                """XT = x[b].T"""
                XT = pool_xt.tile([128, CB, N], MM, tag="XT")
                for nb in range(NB):
                    xrow = emit_a_row_dma(b, nb)
                    emit_a_row_transpose(XT, xrow, nb)
                return XT

            def emit_qk_tile(XT, QT, KT, ob, nh):
                """One q/k output tile: 6 fp32r matmuls + ACT bias-add that
                writes the fp8 (or f32r) QT/KT slice."""
                dest = QT if ob < CB else KT
                dcb = ob % CB
                ps = psmm.tile([128, NH], f32, tag="mm")
                for cb in range(CB):
                    nc.tensor.matmul(
                        ps,
                        W[:, cb, ob * 128 : (ob + 1) * 128],
                        XT[:, cb, nh * NH : (nh + 1) * NH],
                        start=(cb == 0),
                        stop=(cb == CB - 1),
                    )
                nc.scalar.add(
                    dest[:, dcb, nh * NH : (nh + 1) * NH], ps, qkb[:, ob : ob + 1]
                )

            def emit_v_chunk(XT, V, mb):
                """V[mb] = x[mb-block] @ Wv + v_bias (DVE: only DVE/ACT can
                read PSUM, and the bias varies along the free dim)."""
                for c0, cw in ((0, NH), (NH, C - NH)):
                    ps = psmm.tile([128, NH], f32, tag="mm")
                    for cb in range(CB):
                        nc.tensor.matmul(
                            ps[:, :cw],
                            XT[:, cb, mb * 128 : (mb + 1) * 128],
                            W[:, cb, 2 * C + c0 : 2 * C + c0 + cw],
                            start=(cb == 0),
                            stop=(cb == CB - 1),
                        )
                    nc.vector.tensor_tensor(
                        V[:, mb, c0 : c0 + cw], ps[:, :cw],
                        vb[:, c0 : c0 + cw], op=mybir.AluOpType.add,
                    )

            def emit_scores_tile(QT, KT, expT, nh, mb):
                """scoresT tile [m-block, n-half] + exp on ACT -> fp8 expT."""
                nsl = slice(nh * NH, (nh + 1) * NH)
                msl = slice(mb * 128, (mb + 1) * 128)
                ps = psmm.tile([128, NH], f32, tag="mm")
                if fp8:
                    for p in range(CB // 2):
                        nc.tensor.matmul(
                            ps,
                            KT[:, 2 * p : 2 * p + 2, msl],
                            QT[:, 2 * p : 2 * p + 2, nsl],
                            start=(p == 0),
                            stop=(p == CB // 2 - 1),
                            perf_mode=DR,
                        )
                else:
                    for cb in range(CB):
                        nc.tensor.matmul(
                            ps, KT[:, cb, msl], QT[:, cb, nsl],
                            start=(cb == 0), stop=(cb == CB - 1),
                        )
                nc.scalar.activation(
                    expT[:, mb, nsl], ps, mybir.ActivationFunctionType.Exp,
                    scale=SCALE, bias=expb[:, 0:1],
                )

            def emit_denom(expT, nh):
                """denominator (replicated on all partitions) for one n-half"""
                nsl = slice(nh * NH, (nh + 1) * NH)
                dps = psmm.tile([128, NH], f32, tag="mm")
                if fp8:
                    ones_v = ones.rearrange("p (k f) -> p k f", k=2)
                    for p in range(NB // 2):
                        nc.tensor.matmul(
                            dps, ones_v, expT[:, 2 * p : 2 * p + 2, nsl],
                            start=(p == 0), stop=(p == NB // 2 - 1),
                            perf_mode=DR,
                        )
                else:
                    for mb in range(NB):
                        nc.tensor.matmul(
                            dps, ones, expT[:, mb, nsl],
                            start=(mb == 0), stop=(mb == NB - 1),
                        )
                return dps

            def emit_av(V, expT, recips, scrv):
                """OT = (V.T @ expT) * recip, streamed to DRAM scratch
                cb-major so stage_e's first rows unblock early."""
                for cb in range(CB):
                    csl = slice(cb * 128, (cb + 1) * 128)
                    for nh in range(N // NH):
                        nsl = slice(nh * NH, (nh + 1) * NH)
                        ps = psmm.tile([128, NH], f32, tag="mm")
                        if fp8:
                            for p in range(NB // 2):
                                nc.tensor.matmul(
                                    ps,
                                    V[:, 2 * p : 2 * p + 2, csl],
                                    expT[:, 2 * p : 2 * p + 2, nsl],
                                    start=(p == 0),
                                    stop=(p == NB // 2 - 1),
                                    perf_mode=DR,
                                )
                        else:
                            for mb in range(NB):
                                nc.tensor.matmul(
                                    ps, V[:, mb, csl], expT[:, mb, nsl],
                                    start=(mb == 0), stop=(mb == NB - 1),
                                )
                        ot = pool_row.tile([128, NH], MM, tag="row")
                        nc.vector.tensor_tensor(
                            ot, ps, recips[nh], op=mybir.AluOpType.mult
                        )
                        nc.sync.dma_start(scrv[csl, nsl], ot)

            def emit_prow(scr, ib, q):
                """P-row load, issued ~2 row-iterations ahead of its use so
                the DMA (and its queue wait) hides behind PE work."""
                pview = scr.rearrange("(i j) -> i j", j=C)
                prow = pool_row.tile([128, C], MM, tag="row")
                q.dma_start(prow, pview[ib * 128 : (ib + 1) * 128, :])
                return prow

            def emit_e_row(prow, b, ib, q):
                """One row-block of y = P @ proj_w + proj_b."""
                pt4a = pool_pt.tile([128, NH], MM, tag="pt4")
                pt4b = pool_pt.tile([128, NH], MM, tag="pt4")
                transpose_block(
                    prow,
                    [
                        pt4a.rearrange("p (c k) -> p c k", k=128),
                        pt4b[:, 0:256].rearrange("p (c k) -> p c k", k=128),
                    ],
                )
                ps1 = psmm.tile([128, NH], f32, tag="mm")
                ps2 = psmm.tile([128, NH], f32, tag="mm")
                for jb in range(CB):
                    pt = (pt4a if jb < 4 else pt4b)[
                        :, (jb % 4) * 128 : (jb % 4 + 1) * 128
                    ]
                    nc.tensor.matmul(
                        ps1, pt, PW[:, jb, 0:NH],
                        start=(jb == 0), stop=(jb == CB - 1),
                    )
                    nc.tensor.matmul(
                        ps2[:, : C - NH], pt, PW[:, jb, NH:C],
                        start=(jb == 0), stop=(jb == CB - 1),
                    )
                yrow = pool_row.tile([128, C], f32, tag="row")
                nc.vector.tensor_tensor(
                    yrow[:, 0:NH], ps1, pb[:, 0:NH], op=mybir.AluOpType.add
                )
                nc.vector.tensor_tensor(
                    yrow[:, NH:C], ps2[:, : C - NH], pb[:, NH:C],
                    op=mybir.AluOpType.add,
                )
                q.dma_start(y.ap()[b, ib * 128 : (ib + 1) * 128, :], yrow)

            # ---------------- emission schedule ----------------
            import contextlib
            _loop_n = int(os.environ.get("BLIP_LOOP", "0"))
            _loop_ctx = tc.For_i(0, _loop_n, 1) if _loop_n else contextlib.nullcontext()
            _loop_ctx.__enter__()

            # prologue: batch-0 x rows load before the big weight DMAs so the
            # PE starts transposing immediately; qkv_w streams in thirds
            # (q cols, k cols, v cols) so the first QK matmuls start after
            # 1/3 of the weight bytes.
            XT_cur = stage_a(0)
            w_view = qkv_w.rearrange("(cb p) o -> p cb o", p=128)
            pw_view = proj_w.rearrange("(cb p) o -> p cb o", p=128)
            for t in range(3):
                osl = slice(t * C, (t + 1) * C)
                for cb in range(CB):
                    nc.sync.dma_start(W[:, cb, osl], w_view[:, cb, osl])
            for cb in range(CB):
                nc.sync.dma_start(PW[:, cb], pw_view[:, cb])

            def make_qkv(XT):
                QT = pool_qt.tile([128, CB, N], AT, tag="QT")
                KT = pool_kt.tile([128, CB, N], AT, tag="KT")
                V = pool_v.tile([128, NB, C], AT, tag="V")
                return QT, KT, V

            qkv_cur = make_qkv(XT_cur)
            for ob in range(2 * CB):
                for nh in range(N // NH):
                    emit_qk_tile(XT_cur, qkv_cur[0], qkv_cur[1], ob, nh)
            for mb in range(NB):
                emit_v_chunk(XT_cur, qkv_cur[2], mb)

            for b in range(BPC):
                last = b + 1 >= BPC
                QT, KT, V = qkv_cur
                if not last:
                    XT_next = pool_xt.tile([128, CB, N], MM, tag="XT")
                    qkv_next = make_qkv(XT_next)

                # scores (both n-halves) with the next batch's x-row loads
                # and transposes woven in: each row DMA gets a scores tile
                # (~1.3us of PE) of cover before its transposes need it
                expT = pool_expt.tile([128, NB, N], AT, tag="expT")
                rows = [None] * NB
                for i, (nh, mb) in enumerate(
                    [(h, m) for h in range(N // NH) for m in range(NB)]
                ):
                    emit_scores_tile(QT, KT, expT, nh, mb)
                    if not last:
                        if i < NB:
                            rows[i] = emit_a_row_dma(b + 1, i)
                        if 1 <= i <= NB:
                            emit_a_row_transpose(XT_next, rows[i - 1], i - 1)

                # reciprocals right after the denominators, before the qk
                # tiles recycle their PSUM bufs
                recips = []
                for nh in range(N // NH):
                    dps = emit_denom(expT, nh)
                    rb = pool_rb.tile([128, NH], f32, tag="recipB")
                    nc.vector.reciprocal(rb, dps)
                    recips.append(rb)

                if not last:
                    for ob in range(2 * CB):
                        for nh in range(N // NH):
                            emit_qk_tile(XT_next, qkv_next[0], qkv_next[1], ob, nh)

                scr = pool_scr.tile([C * N], MM, tag="scr")
                scrv = scr.rearrange("(c n) -> c n", n=N)
                emit_av(V, expT, recips, scrv)

                # the last body has no v-chunks between e-rows, so the sync
                # ring (ot+prow+y back-to-back) saturates and queue order
                # blocks prow behind all ot writes — give its e-phase DMAs
                # their own ring
                eq = nc.gpsimd if last else nc.sync
                prows = [None] * NB
                prows[0] = emit_prow(scr, 0, eq)
                prows[1] = emit_prow(scr, 1, eq)
                for ib in range(NB):
                    if not last:
                        emit_v_chunk(XT_next, qkv_next[2], ib)
                    emit_e_row(prows[ib], b, ib, eq)
                    if ib + 2 < NB:
                        prows[ib + 2] = emit_prow(scr, ib + 2, eq)

                if not last:
                    XT_cur, qkv_cur = XT_next, qkv_next

            _loop_ctx.__exit__(None, None, None)

    nc.compile()
    return nc


def _get_nc():
    mm_r = os.environ.get("BLIP_MM_DTYPE", "float32r") != "float32"
    # fp8 DoubleRow attention measures rel_err ~2.6e-2 on these inputs —
    # over the 2e-2 gate — so the bf16 core (rel_err 1.6e-3) is the default.
    fp8 = os.environ.get("BLIP_FP8", "0") == "1"
    key = ("nc", mm_r, fp8)
    if key not in _CACHE:
        _CACHE[key] = _build(mm_r, fp8)
    return _CACHE[key]


def kernel(x, qkv_w, qkv_b, proj_w, proj_b, _trace=False, _tmpdir=None):
    x = np.ascontiguousarray(np.asarray(x, dtype=np.float32))
    shared = {
        "qkv_w": np.ascontiguousarray(np.asarray(qkv_w, dtype=np.float32)),
        "qkv_b": np.ascontiguousarray(np.asarray(qkv_b, dtype=np.float32)),
        "proj_w": np.ascontiguousarray(np.asarray(proj_w, dtype=np.float32)),
        "proj_b": np.ascontiguousarray(np.asarray(proj_b, dtype=np.float32)),
    }
    nc = _get_nc()
    in_maps = [
        {"xs": x[c * BPC : (c + 1) * BPC], **shared} for c in range(NCORES)
    ]
    res = run_bass_kernel_spmd(
        nc, in_maps, core_ids=list(range(NCORES)),
        trace=_trace, tmpdir=_tmpdir,
        **({"trace_cores": [0]} if _trace else {}),
    )
    out = np.concatenate([res.results[c]["y"] for c in range(NCORES)], axis=0)
    if _trace:
        return out, res
    return out
